# revision 14
# baseline (speedup 1.0000x reference)
"""Conv2d (32,128,56,56) x (256,128,3,3) pad=1 -> (32,256,56,56) on 8 trn2 cores.

Strategy: data-parallel over batch (4 images/core). On each core the conv is
9 accumulating matmuls per output tile: contraction over C=128 (partition
dim), stationary operand = per-tap weight slab [C=128, O_half=128], moving
operand = shifted window of the zero-padded input rows [C=128, 8 rows x 56].
PSUM accumulates the 9 taps; DVE adds bias while evacuating to SBUF; DMA out.

v2: bf16 operands (FWL halves LDWEIGHTS, rel err ~2e-3 vs 2e-2 gate),
bf16 output (halves store traffic, host upcasts), warmup sized to end when
the first DMA lands, fewer out-pool buffers (shorter semaphore teardown).
"""

import os
import sys

for _p in ("/opt/trn_rl_repo", "/root/.axon_site/_ro/trn_rl_repo"):
    if os.path.isdir(_p) and _p not in sys.path:
        sys.path.insert(0, _p)

import numpy as np

N_CORES = 8
B, C, H, W = 32, 128, 56, 56
O, KH, KW = 256, 3, 3
BPC = B // N_CORES          # images per core
HP, WP = H + 2, W + 2       # padded spatial
ROWS = 8                    # output rows per matmul chunk
NCH = H // ROWS             # chunks per image
NF = ROWS * W               # matmul free dim (448 <= 512 fp32 PSUM bank)

N_WARM = 15                 # PE-warmup matmuls (~5.6us cold, ends ~ first DMA)

_cached_nc = None


def _build_program():
    import concourse.tile as tile
    from concourse import bacc, mybir

    nc = bacc.Bacc(
        "TRN2", target_bir_lowering=False, debug=False, num_devices=N_CORES
    )
    f32 = mybir.dt.float32
    bf16 = mybir.dt.bfloat16

    xp = nc.dram_tensor("xp", (C, BPC, HP, WP), bf16, kind="ExternalInput").ap()
    wt = nc.dram_tensor("wt", (C, O // C, KH * KW, 128), bf16, kind="ExternalInput").ap()
    bias = nc.dram_tensor("bias", (C, O // C), f32, kind="ExternalInput").ap()
    out = nc.dram_tensor("out", (BPC * O, H * W), bf16, kind="ExternalOutput").ap()

    with tile.TileContext(nc) as tc:
        with (
            tc.tile_pool(name="consts", bufs=1) as consts,
            tc.tile_pool(name="xpool", bufs=1) as xpool,
            tc.tile_pool(name="opool", bufs=4) as opool,
            tc.tile_pool(name="psum", bufs=7, space="PSUM") as pspool,
        ):
            # PE prewarm: dummy matmuls on scratch tiles while DMAs stream in,
            # so the HAM clock gate reaches 8/8 before the real matmuls start
            # and stays there until they do.
            warm_x = consts.tile([C, NF], bf16, tag="warm_x")
            nc.gpsimd.memset(warm_x[:], 0.0)
            warm_ps = pspool.tile([128, NF], f32, tag="ps")
            for _ in range(N_WARM):
                nc.tensor.matmul(
                    warm_ps[:], warm_x[:, :128], warm_x[:], start=True, stop=True
                )

            # All loads go on the single sync HWDGE ring in hand-picked FIFO
            # order: the ring runs at full HBM bandwidth with no sharing, so
            # the critical prefix (image-0 band 1 + oh=0 weights) lands first.
            # Image 0 is split in row bands so early chunks start sooner
            # (chunk c reads rows 8c..8c+9, so bands overlap by 2 rows).
            bands = [(0, 12), (10, 34), (32, HP)]
            w_sb = consts.tile([C, O // C, KH * KW, 128], bf16)
            bias_sb = consts.tile([C, O // C], f32)
            x_sbs = []
            for i in range(BPC):
                x_sb = xpool.tile([C, HP, WP], bf16, tag=f"x{i}")
                x_sbs.append(x_sb)
            nc.sync.dma_start(x_sbs[0][:, bands[0][0] : bands[0][1]],
                              xp[:, 0, bands[0][0] : bands[0][1]])
            nc.sync.dma_start(w_sb[:, 0], wt[:, 0])
            nc.sync.dma_start(bias_sb[:], bias[:])
            for r0, r1 in bands[1:]:
                nc.sync.dma_start(x_sbs[0][:, r0:r1], xp[:, 0, r0:r1])
            nc.sync.dma_start(w_sb[:, 1], wt[:, 1])
            for i in range(1, BPC):
                nc.sync.dma_start(x_sbs[i][:], xp[:, i])

            def emit_tile(i, oh, y0, rows, col0):
                nf = rows * W
                ps = pspool.tile([128, NF], f32, tag="ps")
                for t in range(KH * KW):
                    kh, kw = divmod(t, KW)
                    rhs = x_sbs[i][:, y0 + kh : y0 + kh + rows, kw : kw + W]
                    lhsT = w_sb[:, oh, t, :]
                    nc.tensor.matmul(
                        ps[:, :nf], lhsT, rhs,
                        start=(t == 0), stop=(t == KH * KW - 1),
                    )
                o_sb = opool.tile([128, NF], bf16, tag="o")
                nc.vector.tensor_scalar_add(
                    o_sb[:, :nf], ps[:, :nf], bias_sb[:, oh : oh + 1]
                )
                r0 = i * O + oh * 128
                # First few stores ride the otherwise-idle GpSimd ring:
                # keeps the sync ring store-free while image-0 bands
                # and weights are still streaming in (12-20us window).
                eng = nc.gpsimd if (i == 0 and oh == 0 and y0 < 32) else nc.sync
                eng.dma_start(out[r0 : r0 + 128, col0 : col0 + nf], o_sb[:, :nf])

            for i in range(BPC):
                for oh in range(O // C):
                    for ch in range(NCH):
                        last = i == BPC - 1 and oh == O // C - 1 and ch == NCH - 1
                        if not last:
                            emit_tile(i, oh, ch * ROWS, ROWS, ch * NF)
                        else:
                            # Final tile split in two 4-row halves: the last
                            # bias-add + store covers half the bytes and
                            # overlaps the second half's matmuls.
                            h = ROWS // 2
                            emit_tile(i, oh, ch * ROWS, h, ch * NF)
                            emit_tile(i, oh, ch * ROWS + h, h, ch * NF + h * W)
    nc.compile()
    return nc


def _get_program():
    global _cached_nc
    if _cached_nc is None:
        _cached_nc = _build_program()
    return _cached_nc


def _prep_inputs(x, kernels, biases):
    """Host-side shard + layout prep. Returns list of per-core input maps."""
    import ml_dtypes

    bf16 = np.dtype(ml_dtypes.bfloat16)
    x = np.ascontiguousarray(x, dtype=np.float32)
    kernels = np.ascontiguousarray(kernels, dtype=np.float32)
    biases = np.ascontiguousarray(biases, dtype=np.float32)

    xpad = np.zeros((B, C, HP, WP), dtype=np.float32)
    xpad[:, :, 1 : H + 1, 1 : W + 1] = x

    # wt[c, oh, t, o'] = kernels[oh*128 + o', c, kh, kw]
    wt = np.ascontiguousarray(
        kernels.transpose(1, 2, 3, 0)
        .reshape(C, KH * KW, O // C, 128)
        .transpose(0, 2, 1, 3)
    ).astype(bf16)
    # bias_sb[o', h] = biases[h*128 + o']
    bias2 = np.ascontiguousarray(biases.reshape(O // C, C).T)

    in_maps = []
    for core in range(N_CORES):
        xc = np.ascontiguousarray(
            xpad[core * BPC : (core + 1) * BPC].transpose(1, 0, 2, 3)
        ).astype(bf16)
        in_maps.append({"xp": xc, "wt": wt, "bias": bias2})
    return in_maps


def _run(in_maps, trace=False, **kw):
    from concourse.bass_utils import run_bass_kernel_spmd

    nc = _get_program()
    return run_bass_kernel_spmd(
        nc, in_maps, core_ids=list(range(N_CORES)), trace=trace, **kw
    )


def kernel(x, kernels, biases):
    res = _run(_prep_inputs(x, kernels, biases))
    outs = [
        r["out"].astype(np.float32).reshape(BPC, O, H, W) for r in res.results
    ]
    return np.concatenate(outs, axis=0)


# revision 16
# speedup vs baseline: 1.0105x; 1.0105x over previous
"""Conv2d (32,128,56,56) x (256,128,3,3) pad=1 -> (32,256,56,56) on 8 trn2 cores.

Strategy: data-parallel over batch (4 images/core). On each core the conv is
9 accumulating matmuls per output tile: contraction over C=128 (partition
dim), stationary operand = per-tap weight slab [C=128, O_half=128], moving
operand = shifted window of the zero-padded input rows [C=128, 8 rows x 56].
PSUM accumulates the 9 taps; DVE adds bias while evacuating to SBUF; DMA out.

v2: bf16 operands (FWL halves LDWEIGHTS, rel err ~2e-3 vs 2e-2 gate),
bf16 output (halves store traffic, host upcasts), warmup sized to end when
the first DMA lands, fewer out-pool buffers (shorter semaphore teardown).
"""

import os
import sys

for _p in ("/opt/trn_rl_repo", "/root/.axon_site/_ro/trn_rl_repo"):
    if os.path.isdir(_p) and _p not in sys.path:
        sys.path.insert(0, _p)

import numpy as np

N_CORES = 8
B, C, H, W = 32, 128, 56, 56
O, KH, KW = 256, 3, 3
BPC = B // N_CORES          # images per core
HP, WP = H + 2, W + 2       # padded spatial
ROWS = 8                    # output rows per matmul chunk
NCH = H // ROWS             # chunks per image
NF = ROWS * W               # matmul free dim (448 <= 512 fp32 PSUM bank)

N_WARM = 15                 # PE-warmup matmuls (~5.6us cold, ends ~ first DMA)

_cached_nc = None


def _build_program():
    import concourse.tile as tile
    from concourse import bacc, mybir

    nc = bacc.Bacc(
        "TRN2", target_bir_lowering=False, debug=False, num_devices=N_CORES
    )
    f32 = mybir.dt.float32
    bf16 = mybir.dt.bfloat16

    xp = nc.dram_tensor("xp", (C, BPC, HP, WP), bf16, kind="ExternalInput").ap()
    wt = nc.dram_tensor("wt", (C, O // C, KH * KW, 128), bf16, kind="ExternalInput").ap()
    bias = nc.dram_tensor("bias", (C, O // C), f32, kind="ExternalInput").ap()
    out = nc.dram_tensor("out", (BPC * O, H * W), bf16, kind="ExternalOutput").ap()

    with tile.TileContext(nc) as tc:
        with (
            tc.tile_pool(name="consts", bufs=1) as consts,
            tc.tile_pool(name="xpool", bufs=1) as xpool,
            tc.tile_pool(name="opool", bufs=4) as opool,
            tc.tile_pool(name="psum", bufs=7, space="PSUM") as pspool,
        ):
            # PE prewarm: dummy matmuls on scratch tiles while DMAs stream in,
            # so the HAM clock gate reaches 8/8 before the real matmuls start
            # and stays there until they do.
            warm_x = consts.tile([C, NF], bf16, tag="warm_x")
            nc.gpsimd.memset(warm_x[:], 0.0)
            warm_ps = pspool.tile([128, NF], f32, tag="warm_ps", bufs=1)
            for _ in range(N_WARM):
                nc.tensor.matmul(
                    warm_ps[:], warm_x[:, :128], warm_x[:], start=True, stop=True
                )

            # All loads go on the single sync HWDGE ring in hand-picked FIFO
            # order: the ring runs at full HBM bandwidth with no sharing, so
            # the critical prefix (image-0 band 1 + oh=0 weights) lands first.
            # Image 0 is split in row bands so early chunks start sooner
            # (chunk c reads rows 8c..8c+9, so bands overlap by 2 rows).
            bands = [(0, 12), (10, 34), (32, HP)]
            w_sb = consts.tile([C, O // C, KH * KW, 128], bf16)
            bias_sb = consts.tile([C, O // C], f32)
            x_sbs = []
            for i in range(BPC):
                x_sb = xpool.tile([C, HP, WP], bf16, tag=f"x{i}")
                x_sbs.append(x_sb)
            nc.sync.dma_start(x_sbs[0][:, bands[0][0] : bands[0][1]],
                              xp[:, 0, bands[0][0] : bands[0][1]])
            nc.sync.dma_start(w_sb[:, 0], wt[:, 0])
            # Tiny bias load rides the GpSimd ring: frees one ~0.7us issue
            # slot on the Sync FIFO so band B starts streaming sooner.
            nc.gpsimd.dma_start(bias_sb[:], bias[:])
            for r0, r1 in bands[1:]:
                nc.sync.dma_start(x_sbs[0][:, r0:r1], xp[:, 0, r0:r1])
            nc.sync.dma_start(w_sb[:, 1], wt[:, 1])
            for i in range(1, BPC):
                nc.sync.dma_start(x_sbs[i][:], xp[:, i])

            for i in range(BPC):
                for oh in range(O // C):
                    for ch in range(NCH):
                        y0 = ch * ROWS
                        ps = pspool.tile([128, NF], f32)
                        for t in range(KH * KW):
                            kh, kw = divmod(t, KW)
                            rhs = x_sbs[i][:, y0 + kh : y0 + kh + ROWS, kw : kw + W]
                            lhsT = w_sb[:, oh, t, :]
                            nc.tensor.matmul(
                                ps[:], lhsT, rhs,
                                start=(t == 0), stop=(t == KH * KW - 1),
                            )
                        o_sb = opool.tile([128, NF], bf16)
                        nc.vector.tensor_scalar_add(
                            o_sb[:], ps[:], bias_sb[:, oh : oh + 1]
                        )
                        r0 = i * O + oh * 128
                        # First few stores ride the otherwise-idle GpSimd ring:
                        # keeps the sync ring store-free while image-0 bands
                        # and weights are still streaming in (12-20us window).
                        eng = nc.gpsimd if (i == 0 and oh == 0 and ch < 4) else nc.sync
                        eng.dma_start(
                            out[r0 : r0 + 128, ch * NF : (ch + 1) * NF], o_sb[:]
                        )
    nc.compile()
    return nc


def _get_program():
    global _cached_nc
    if _cached_nc is None:
        _cached_nc = _build_program()
    return _cached_nc


def _prep_inputs(x, kernels, biases):
    """Host-side shard + layout prep. Returns list of per-core input maps."""
    import ml_dtypes

    bf16 = np.dtype(ml_dtypes.bfloat16)
    x = np.ascontiguousarray(x, dtype=np.float32)
    kernels = np.ascontiguousarray(kernels, dtype=np.float32)
    biases = np.ascontiguousarray(biases, dtype=np.float32)

    xpad = np.zeros((B, C, HP, WP), dtype=np.float32)
    xpad[:, :, 1 : H + 1, 1 : W + 1] = x

    # wt[c, oh, t, o'] = kernels[oh*128 + o', c, kh, kw]
    wt = np.ascontiguousarray(
        kernels.transpose(1, 2, 3, 0)
        .reshape(C, KH * KW, O // C, 128)
        .transpose(0, 2, 1, 3)
    ).astype(bf16)
    # bias_sb[o', h] = biases[h*128 + o']
    bias2 = np.ascontiguousarray(biases.reshape(O // C, C).T)

    in_maps = []
    for core in range(N_CORES):
        xc = np.ascontiguousarray(
            xpad[core * BPC : (core + 1) * BPC].transpose(1, 0, 2, 3)
        ).astype(bf16)
        in_maps.append({"xp": xc, "wt": wt, "bias": bias2})
    return in_maps


def _run(in_maps, trace=False, **kw):
    from concourse.bass_utils import run_bass_kernel_spmd

    nc = _get_program()
    return run_bass_kernel_spmd(
        nc, in_maps, core_ids=list(range(N_CORES)), trace=trace, **kw
    )


def kernel(x, kernels, biases):
    res = _run(_prep_inputs(x, kernels, biases))
    outs = [
        r["out"].astype(np.float32).reshape(BPC, O, H, W) for r in res.results
    ]
    return np.concatenate(outs, axis=0)


# revision 17
# speedup vs baseline: 1.0200x; 1.0093x over previous
"""Conv2d (32,128,56,56) x (256,128,3,3) pad=1 -> (32,256,56,56) on 8 trn2 cores.

Strategy: data-parallel over batch (4 images/core). On each core the conv is
9 accumulating matmuls per output tile: contraction over C=128 (partition
dim), stationary operand = per-tap weight slab [C=128, O_half=128], moving
operand = shifted window of the zero-padded input rows [C=128, 8 rows x 56].
PSUM accumulates the 9 taps; DVE adds bias while evacuating to SBUF; DMA out.

v2: bf16 operands (FWL halves LDWEIGHTS, rel err ~2e-3 vs 2e-2 gate),
bf16 output (halves store traffic, host upcasts), warmup sized to end when
the first DMA lands, fewer out-pool buffers (shorter semaphore teardown).
"""

import os
import sys

for _p in ("/opt/trn_rl_repo", "/root/.axon_site/_ro/trn_rl_repo"):
    if os.path.isdir(_p) and _p not in sys.path:
        sys.path.insert(0, _p)

import numpy as np

N_CORES = 8
B, C, H, W = 32, 128, 56, 56
O, KH, KW = 256, 3, 3
BPC = B // N_CORES          # images per core
HP, WP = H + 2, W + 2       # padded spatial
ROWS = 8                    # output rows per matmul chunk
NCH = H // ROWS             # chunks per image
NF = ROWS * W               # matmul free dim (448 <= 512 fp32 PSUM bank)

N_WARM = 13                 # PE-warmup matmuls (~4.8us cold, ends ~ first DMA)

_cached_nc = None


def _build_program():
    import concourse.tile as tile
    from concourse import bacc, mybir

    nc = bacc.Bacc(
        "TRN2", target_bir_lowering=False, debug=False, num_devices=N_CORES
    )
    f32 = mybir.dt.float32
    bf16 = mybir.dt.bfloat16

    xp = nc.dram_tensor("xp", (C, BPC, HP, WP), bf16, kind="ExternalInput").ap()
    wt = nc.dram_tensor("wt", (C, O // C, KH * KW, 128), bf16, kind="ExternalInput").ap()
    bias = nc.dram_tensor("bias", (C, O // C), f32, kind="ExternalInput").ap()
    out = nc.dram_tensor("out", (BPC * O, H * W), bf16, kind="ExternalOutput").ap()

    with tile.TileContext(nc) as tc:
        with (
            tc.tile_pool(name="consts", bufs=1) as consts,
            tc.tile_pool(name="xpool", bufs=1) as xpool,
            tc.tile_pool(name="opool", bufs=4) as opool,
            tc.tile_pool(name="psum", bufs=7, space="PSUM") as pspool,
        ):
            # PE prewarm: dummy matmuls on scratch tiles while DMAs stream in,
            # so the HAM clock gate reaches 8/8 before the real matmuls start
            # and stays there until they do.
            warm_x = consts.tile([C, NF], bf16, tag="warm_x")
            nc.gpsimd.memset(warm_x[:], 0.0)
            warm_ps = pspool.tile([128, NF], f32, tag="warm_ps", bufs=1)
            for _ in range(N_WARM):
                nc.tensor.matmul(
                    warm_ps[:], warm_x[:, :128], warm_x[:], start=True, stop=True
                )

            # All loads go on the single sync HWDGE ring in hand-picked FIFO
            # order: the ring runs at full HBM bandwidth with no sharing, so
            # the critical prefix (image-0 band 1 + oh=0 weights) lands first.
            # Image 0 is split in row bands so early chunks start sooner
            # (chunk c reads rows 8c..8c+9, so bands overlap by 2 rows).
            bands = [(0, 12), (10, 34), (32, HP)]
            w_sb = consts.tile([C, O // C, KH * KW, 128], bf16)
            bias_sb = consts.tile([C, O // C], f32)
            x_sbs = []
            for i in range(BPC):
                x_sb = xpool.tile([C, HP, WP], bf16, tag=f"x{i}")
                x_sbs.append(x_sb)
            nc.sync.dma_start(x_sbs[0][:, bands[0][0] : bands[0][1]],
                              xp[:, 0, bands[0][0] : bands[0][1]])
            nc.sync.dma_start(w_sb[:, 0], wt[:, 0])
            # Tiny bias load rides the GpSimd ring: frees one ~0.7us issue
            # slot on the Sync FIFO so band B starts streaming sooner.
            nc.gpsimd.dma_start(bias_sb[:], bias[:])
            for r0, r1 in bands[1:]:
                nc.sync.dma_start(x_sbs[0][:, r0:r1], xp[:, 0, r0:r1])
            nc.sync.dma_start(w_sb[:, 1], wt[:, 1])
            for i in range(1, BPC):
                nc.sync.dma_start(x_sbs[i][:], xp[:, i])

            for i in range(BPC):
                for oh in range(O // C):
                    for ch in range(NCH):
                        y0 = ch * ROWS
                        ps = pspool.tile([128, NF], f32)
                        for t in range(KH * KW):
                            kh, kw = divmod(t, KW)
                            rhs = x_sbs[i][:, y0 + kh : y0 + kh + ROWS, kw : kw + W]
                            lhsT = w_sb[:, oh, t, :]
                            nc.tensor.matmul(
                                ps[:], lhsT, rhs,
                                start=(t == 0), stop=(t == KH * KW - 1),
                            )
                        o_sb = opool.tile([128, NF], bf16)
                        nc.vector.tensor_scalar_add(
                            o_sb[:], ps[:], bias_sb[:, oh : oh + 1]
                        )
                        r0 = i * O + oh * 128
                        # First few stores ride the otherwise-idle GpSimd ring:
                        # keeps the sync ring store-free while image-0 bands
                        # and weights are still streaming in (12-20us window).
                        eng = nc.gpsimd if (i == 0 and oh == 0 and ch < 4) else nc.sync
                        eng.dma_start(
                            out[r0 : r0 + 128, ch * NF : (ch + 1) * NF], o_sb[:]
                        )
    nc.compile()
    return nc


def _get_program():
    global _cached_nc
    if _cached_nc is None:
        _cached_nc = _build_program()
    return _cached_nc


def _prep_inputs(x, kernels, biases):
    """Host-side shard + layout prep. Returns list of per-core input maps."""
    import ml_dtypes

    bf16 = np.dtype(ml_dtypes.bfloat16)
    x = np.ascontiguousarray(x, dtype=np.float32)
    kernels = np.ascontiguousarray(kernels, dtype=np.float32)
    biases = np.ascontiguousarray(biases, dtype=np.float32)

    xpad = np.zeros((B, C, HP, WP), dtype=np.float32)
    xpad[:, :, 1 : H + 1, 1 : W + 1] = x

    # wt[c, oh, t, o'] = kernels[oh*128 + o', c, kh, kw]
    wt = np.ascontiguousarray(
        kernels.transpose(1, 2, 3, 0)
        .reshape(C, KH * KW, O // C, 128)
        .transpose(0, 2, 1, 3)
    ).astype(bf16)
    # bias_sb[o', h] = biases[h*128 + o']
    bias2 = np.ascontiguousarray(biases.reshape(O // C, C).T)

    in_maps = []
    for core in range(N_CORES):
        xc = np.ascontiguousarray(
            xpad[core * BPC : (core + 1) * BPC].transpose(1, 0, 2, 3)
        ).astype(bf16)
        in_maps.append({"xp": xc, "wt": wt, "bias": bias2})
    return in_maps


def _run(in_maps, trace=False, **kw):
    from concourse.bass_utils import run_bass_kernel_spmd

    nc = _get_program()
    return run_bass_kernel_spmd(
        nc, in_maps, core_ids=list(range(N_CORES)), trace=trace, **kw
    )


def kernel(x, kernels, biases):
    res = _run(_prep_inputs(x, kernels, biases))
    outs = [
        r["out"].astype(np.float32).reshape(BPC, O, H, W) for r in res.results
    ]
    return np.concatenate(outs, axis=0)
